# revision 1
# baseline (speedup 1.0000x reference)
"""Trainium2 Bass kernel for a DeepSeek-MLA-style differential-attention layer.

Sharding: tensor-parallel over heads. 16 heads / 8 cores = 2 heads per core.
Launch A computes the low-rank projections seq-sharded (each core does 1/8 of
the positions); the host gathers and rebroadcasts them; launch B computes each
core's 2 heads of attention plus a partial output projection; the host sums
the 8 partials (the "all-reduce after wo").

Everything on-chip is fp16 (PSUM accumulation stays fp32): halves DMA traffic
and unlocks the DVE 2x/4x perf modes and PE fast-weight-load.

Layouts are feature-major ([feature, seq]) end to end so every matmul
contraction lands on the partition dimension with no on-device transposes:
  - scores are computed k-major  sT[kpos, qpos]
  - softmax partition-sums use a ones-vector matmul, two z rows packed into
    one PSUM bank via column tiling (tile_position) so they share a PE slot
  - attn @ v consumes the k-major exp tiles directly (lhsT = v[kpos, dv])
Causal structure: fully-masked k-major blocks are skipped; partially-masked
(diagonal) blocks are narrowed to their active column range.
RoPE pairs are de-interleaved into (real | imag) blocks by permuting weight
rows on the host.
"""

import math

import numpy as np

import concourse.mybir as mybir
import concourse.tile as tile
from concourse import bacc
from concourse.bass_utils import run_bass_kernel_spmd

F32 = mybir.dt.float32
F16 = mybir.dt.float16

DIM = 2048
NH = 16
QLR = 768
KVLR = 512
DN = 128
DR = 64
DV = 128
QKH = DN + DR          # 192
H = QKH // 2           # 96
SEQ = 2048
N_CORES = 8
HPC = NH // N_CORES    # heads per core = 2
LAYER_IDX = 3
LAMBDA_INIT = 0.8 - 0.6 * math.exp(-0.3 * LAYER_IDX)
SCALE = QKH ** -0.5
MAX_SCORE = 100.0

KP = 128               # kpos block (partition dim of k-major score tiles)
QP = 512               # qpos block (free dim of score tiles)
NKB = SEQ // KP        # 16 kpos blocks
NQB = SEQ // QP        # 4 qpos blocks

_rope_block_perm = np.concatenate([np.arange(0, DR, 2), np.arange(1, DR, 2)])


SW = SEQ // N_CORES      # per-core seq slice width in launch 1 (256)


def _prep_shared(x, wq_a, wkv_a, freqs_cos, freqs_sin, mask):
    """Host-side layout prep shared by all cores (all cheap reshapes)."""
    x0 = np.ascontiguousarray(x.reshape(SEQ, DIM).astype(np.float32))

    # xT tiled partition-major: [128, DIM//128, SEQ]
    xT = x0.T                                        # [DIM, SEQ]
    xT_t = np.ascontiguousarray(
        xT.reshape(DIM // 128, 128, SEQ).transpose(1, 0, 2)).astype(np.float16)

    # wq_a and wkv_a rows stacked: [0:768 qa | 768:1280 kv | 1280:1344 k_pe]
    # (k_pe rows de-interleaved); lhsT layout tiled [128, 16, 1344].
    perm = np.concatenate([np.arange(KVLR), KVLR + _rope_block_perm])
    wab = np.vstack([wq_a, wkv_a[perm]]).T.astype(np.float16)  # [DIM, 1344]
    wab_t = np.ascontiguousarray(
        wab.reshape(DIM // 128, 128, QLR + KVLR + DR).transpose(1, 0, 2))

    # cos/sin transposed and stacked twice: [64, SEQ]
    cosT = freqs_cos.T.astype(np.float16)            # [32, SEQ]
    sinT = freqs_sin.T.astype(np.float16)
    cosT2 = np.ascontiguousarray(np.vstack([cosT, cosT]))
    sinT2 = np.ascontiguousarray(np.vstack([sinT, sinT]))

    ones = np.ones((128, 1), np.float16)

    # Mask block analysis (k-major blocks: [KP kpos, QP qpos]).
    #   kind 0 = fully allowed, 1 = fully masked (skip), 2 = partial.
    # Partial blocks are narrowed to columns [d, QP) where the leading d
    # columns are fully masked; the narrowed exp-mask pattern is deduped and
    # stored at a column offset in one flat [KP, total] array.
    mask = np.asarray(mask, np.float32)
    block_kind = np.zeros((NKB, NQB), np.int8)
    block_d = np.zeros((NKB, NQB), np.int32)
    block_moff = -np.ones((NKB, NQB), np.int64)
    uniq = {}
    pat_list = []
    for n in range(NQB):
        for m in range(NKB):
            blk = mask[n * QP:(n + 1) * QP, m * KP:(m + 1) * KP].T  # [KP, QP]
            if np.all(blk == 0.0):
                block_kind[m, n] = 0
            elif np.all(blk <= -1e8):
                block_kind[m, n] = 1
            else:
                block_kind[m, n] = 2
                dead = np.all(blk <= -1e8, axis=0)       # per-column
                d = 0
                while d < QP and dead[d]:
                    d += 1
                block_d[m, n] = d
                e = np.exp(np.maximum(blk[:, d:], -200.0)).astype(np.float16)
                e = np.hstack([e, e])          # doubled: applies to (s1|s2)
                key = e.tobytes()
                if key not in uniq:
                    uniq[key] = len(pat_list)
                    pat_list.append(e)
                block_moff[m, n] = uniq[key]
    # narrowing correctness: the first unskipped block of every column must
    # start at column 0 so the PSUM accumulation base covers the block
    for n in range(NQB):
        m_list = [m for m in range(NKB) if block_kind[m, n] != 1]
        assert m_list, f"fully masked q-block {n}"
        first = m_list[0]
        assert block_kind[first, n] == 0 or block_d[first, n] == 0, \
            "first block of a q-column must span all columns"
    offs = []
    off = 0
    for e in pat_list:
        offs.append(off)
        off += e.shape[1]
    emask = (np.concatenate(pat_list, axis=1) if pat_list
             else np.zeros((KP, 1), np.float16))
    # patch pattern-id -> flat column offset
    moff = np.full((NKB, NQB), 0, np.int64)
    for n in range(NQB):
        for m in range(NKB):
            if block_moff[m, n] >= 0:
                moff[m, n] = offs[int(block_moff[m, n])]

    shared_a = dict(wab=wab_t.reshape(128, -1))
    x_slices = [np.ascontiguousarray(
        xT_t[:, :, c * SW:(c + 1) * SW]).reshape(128, -1)
        for c in range(N_CORES)]
    # shared part of the launch-B const pack: [cos|sin | emask | ones]
    # (cos/sin side by side on partitions 0:64 — DVE tensor_tensor needs
    # equal base partitions for both SBUF operands)
    cossin = np.zeros((128, 2 * SEQ), np.float16)
    cossin[0:64, 0:SEQ] = cosT2
    cossin[0:64, SEQ:] = sinT2
    cst_shared = np.hstack([cossin, emask, ones.astype(np.float16)])
    shared_b = dict(cst_shared=cst_shared)
    return (shared_a, x_slices, shared_b,
            block_kind, block_d, moff, emask.shape[1])


def _prep_core(core, wq_b, wkv_b, wo, lam):
    """Per-core weight shards (heads 2*core, 2*core+1)."""
    h0, h1 = HPC * core, HPC * core + 1

    # wq_b rows, permuted: [h0 nope | h1 nope | h0r h1r h0i h1i], SCALE folded.
    def q_rows(h):
        base = h * QKH
        nope = np.arange(base, base + DN)
        rope = base + DN + _rope_block_perm
        return nope, rope
    n0, r0 = q_rows(h0)
    n1, r1 = q_rows(h1)
    rows = np.concatenate([n0, n1, r0[:32], r1[:32], r0[32:], r1[32:]])
    wq_bT = (wq_b[rows] * SCALE).T.astype(np.float16)       # [QLR, 384]
    wq_bT_t = np.ascontiguousarray(
        wq_bT.reshape(QLR // 128, 128, HPC * QKH).transpose(1, 0, 2))

    # wkv_b rows: [h0 knope | h0 v | h1 knope | h1 v]
    def kv_rows(h):
        base = h * (DN + DV)
        return np.arange(base, base + DN), np.arange(base + DN, base + DN + DV)
    kn0, v0 = kv_rows(h0)
    kn1, v1 = kv_rows(h1)
    rows = np.concatenate([kn0, v0, kn1, v1])
    wkv_bT = wkv_b[rows].T.astype(np.float16)               # [KVLR, 512]
    wkv_bT_t = np.ascontiguousarray(
        wkv_bT.reshape(KVLR // 128, 128, HPC * (DN + DV)).transpose(1, 0, 2))

    # wo columns for these heads, (1 - LAMBDA_INIT) folded.
    cols = np.concatenate([np.arange(h0 * DV, (h0 + 1) * DV),
                           np.arange(h1 * DV, (h1 + 1) * DV)])
    woT = (wo[:, cols] * (1.0 - LAMBDA_INIT)).T.astype(np.float16)  # [256, DIM]
    woT_t = np.ascontiguousarray(
        woT.reshape(2, 128, DIM).transpose(1, 0, 2))

    return dict(wq_bT=wq_bT_t.reshape(128, -1),
                wkv_bT=wkv_bT_t.reshape(128, -1),
                woT_flat=woT_t.reshape(128, -1))


def _host_prep(inputs):
    lam = (math.exp(float(np.dot(inputs["lambda_q_nope"],
                                 inputs["lambda_k_nope"])))
           - math.exp(float(np.dot(inputs["lambda_q_rope"],
                                   inputs["lambda_k_rope"])))
           + LAMBDA_INIT)
    (shared_a, x_slices, shared_b, block_kind, block_d, block_moff,
     emask_cols) = _prep_shared(
        inputs["x"], inputs["wq_a"], inputs["wkv_a"],
        inputs["freqs_cos"], inputs["freqs_sin"], inputs["mask"])
    in_maps_a = [dict(shared_a, xTs=x_slices[c]) for c in range(N_CORES)]
    in_maps_b = []
    for c in range(N_CORES):
        pc = _prep_core(c, inputs["wq_b"], inputs["wkv_b"],
                        inputs["wo"], lam)
        m = dict(wq_bT=pc["wq_bT"], wkv_bT=pc["wkv_bT"],
                 cst=np.hstack([shared_b["cst_shared"], pc["woT_flat"]]))
        in_maps_b.append(m)
    binfo = (block_kind, block_d, block_moff, emask_cols, lam)
    return in_maps_a, in_maps_b, binfo


def _build_a(nc, repeat=1):
    """Launch 1: seq-sharded low-rank projections. Each core's xTs input is
    its own 256-column slice of x^T; outputs are that slice of qa/kv/kpe."""
    KD = DIM // 128
    KQ = QLR // 128
    KV = KVLR // 128
    FTOT = QLR + KVLR + DR           # 1344 output features
    xTs = nc.dram_tensor("xTs", [128, KD * SW], F16, kind="ExternalInput") \
        .ap().rearrange("p (k s) -> p k s", k=KD)
    wab = nc.dram_tensor("wab", [128, KD * FTOT], F16,
                         kind="ExternalInput").ap() \
        .rearrange("p (k m) -> p k m", k=KD)
    qa_s = nc.dram_tensor("qa_s", [128, KQ * SW], F16, kind="ExternalOutput") \
        .ap()
    kv_s = nc.dram_tensor("kv_s", [128, KV * SW], F16, kind="ExternalOutput") \
        .ap()
    kpe_s = nc.dram_tensor("kpe_s", [DR, SW], F16, kind="ExternalOutput").ap()

    # weight chunks (column ranges of the 1344 features), each a 128-multiple
    CH = [(0, 512), (512, 1024), (1024, FTOT)]
    with tile.TileContext(nc) as tc:
        for _rep in range(repeat):
            with tc.tile_pool(name="pa", bufs=1) as pa, \
                 tc.tile_pool(name="paw", bufs=2) as paw, \
                 tc.tile_pool(name="psA", bufs=4, space="PSUM") as psA:
                xt = pa.tile([128, KD, SW], F16, tag="xt")
                nc.sync.dma_start(out=xt, in_=xTs)
                qa_st = pa.tile([128, KQ, SW], F16, tag="qa_st")
                kv_st = pa.tile([128, KV, SW], F16, tag="kv_st")
                kpe_st = pa.tile([DR, SW], F16, tag="kpe_st")
                for c0, c1 in CH:
                    wch = paw.tile([128, KD, 512], F16, tag="wch")
                    nc.sync.dma_start(out=wch[:, :, :c1 - c0],
                                      in_=wab[:, :, c0:c1])
                    for mb in range((c1 - c0 + 127) // 128):
                        m = (c0 + mb * 128) // 128     # global 128-row chunk
                        mwid = min(128, FTOT - (c0 + mb * 128))
                        ps = psA.tile([128, SW], F32, tag="psA")
                        for k in range(KD):
                            nc.tensor.matmul(
                                ps[:mwid],
                                wch[:, k, mb * 128:mb * 128 + mwid],
                                xt[:, k, :],
                                start=(k == 0), stop=(k == KD - 1))
                        if m < KQ:
                            nc.any.tensor_copy(qa_st[:, m, :], ps[:mwid])
                        elif m < KQ + KV:
                            nc.any.tensor_copy(kv_st[:, m - KQ, :], ps[:mwid])
                        else:
                            nc.any.tensor_copy(kpe_st, ps[:DR])
                nc.sync.dma_start(out=qa_s, in_=qa_st)
                nc.sync.dma_start(out=kv_s, in_=kv_st)
                nc.sync.dma_start(out=kpe_s, in_=kpe_st)


def _build_be(nc, binfo, repeat=1):
    block_kind, block_d, block_moff, emask_cols, lam = binfo
    NM = max(emask_cols, 1)
    KQ = QLR // 128          # 6 over QLR
    KV = KVLR // 128         # 4 over KVLR
    MM = mybir.AluOpType
    Exp = mybir.ActivationFunctionType.Exp

    qa_d = nc.dram_tensor("qa_d", [128, KQ * SEQ], F16,
                          kind="ExternalInput").ap() \
        .rearrange("p (k s) -> p k s", k=KQ)
    kv_d = nc.dram_tensor("kv_d", [128, KV * SEQ], F16,
                          kind="ExternalInput").ap() \
        .rearrange("p (k s) -> p k s", k=KV)
    kpe_d = nc.dram_tensor("kpe_d", [DR, SEQ], F16, kind="ExternalInput").ap()
    wq_bT = nc.dram_tensor("wq_bT", [128, KQ * HPC * QKH], F16,
                           kind="ExternalInput").ap() \
        .rearrange("p (k m) -> p k m", k=KQ)
    wkv_bT = nc.dram_tensor("wkv_bT", [128, KV * HPC * (DN + DV)], F16,
                            kind="ExternalInput").ap() \
        .rearrange("p (k m) -> p k m", k=KV)
    CW = 2 * SEQ + NM + 1 + 2 * DIM  # [cos|sin | emask | ones | woT]
    cst_d = nc.dram_tensor("cst", [128, CW], F16, kind="ExternalInput").ap()
    yT = nc.dram_tensor("yT", [DIM, SEQ], F16, kind="ExternalOutput").ap()

    with tile.TileContext(nc) as tc:
      with tc.tile_pool(name="shared", bufs=1) as pp:
        ct = pp.tile([128, CW], F16, tag="cst")
        nc.sync.dma_start(out=ct[:, 0:2 * SEQ + NM + 1],
                          in_=cst_d[:, 0:2 * SEQ + NM + 1])
        cost = ct[0:64, 0:SEQ]
        sint = ct[0:64, SEQ:2 * SEQ]
        emt = ct[:, 2 * SEQ:2 * SEQ + NM]
        onest = ct[:, 2 * SEQ + NM:2 * SEQ + NM + 1]
        wot = ct[:, 2 * SEQ + NM + 1:].rearrange("p (k m) -> p k m", k=2)

        for _rep in range(repeat):
            # ======== persistent head tensors ========
            with tc.tile_pool(name="heads", bufs=1) as hp:
                qnope = [hp.tile([128, SEQ], F16, name=f"qn{h}", tag=f"qn{h}")
                         for h in range(HPC)]
                knope = [hp.tile([128, SEQ], F16, name=f"kn{h}", tag=f"kn{h}")
                        for h in range(HPC)]
                vv = [hp.tile([128, NKB, DV], F16, name=f"v{h}", tag=f"v{h}")
                      for h in range(HPC)]
                vl = [hp.tile([128, NKB, DV], F16, name=f"vl{h}", tag=f"vl{h}")
                      for h in range(HPC)]
                outT = [hp.tile([128, SEQ], F16, name=f"o{h}", tag=f"o{h}")
                        for h in range(HPC)]
                qrope = hp.tile([128, SEQ], F16, tag="qrope")
                kpe = hp.tile([DR, SEQ], F16, tag="kpe")
                q2 = [hp.tile([H, SEQ], F16, name=f"q2{h}", tag=f"q2{h}")
                      for h in range(HPC)]
                k2 = [hp.tile([H, SEQ], F16, name=f"k2{h}", tag=f"k2{h}")
                      for h in range(HPC)]
                qa_t = hp.tile([128, KQ, SEQ], F16, tag="qa_t")
                kv_t = hp.tile([128, KV, SEQ], F16, tag="kv_t")
                qa_n = [qa_t[:, :, n * QP:(n + 1) * QP] for n in range(NQB)]
                kv_n = [kv_t[:, :, n * QP:(n + 1) * QP] for n in range(NQB)]

                wqb = hp.tile([128, KQ, HPC * QKH], F16, tag="wqb")
                wkvb = hp.tile([128, KV, HPC * (DN + DV)], F16, tag="wkvb")
                if True:
                    # load order = need order: first matmul wants wqb + qa;
                    # n-block slices land just before their projections fire
                    nc.sync.dma_start(out=wqb, in_=wq_bT)
                    nc.sync.dma_start(out=qa_t[:, :, 0:QP],
                                      in_=qa_d[:, :, 0:QP])
                    nc.sync.dma_start(out=kpe, in_=kpe_d)
                    nc.sync.dma_start(out=wkvb, in_=wkv_bT)
                    nc.sync.dma_start(out=kv_t[:, :, 0:QP],
                                      in_=kv_d[:, :, 0:QP])
                    for nn in range(1, NQB):
                        nsl = slice(nn * QP, (nn + 1) * QP)
                        nc.sync.dma_start(out=qa_t[:, :, nsl],
                                          in_=qa_d[:, :, nsl])
                        nc.sync.dma_start(out=kv_t[:, :, nsl],
                                          in_=kv_d[:, :, nsl])
                    nc.sync.dma_start(out=ct[:, 2 * SEQ + NM + 1:],
                                      in_=cst_d[:, 2 * SEQ + NM + 1:])

                    # ==== phase B1: qrope projections + rope (DVE) ====
                    with tc.tile_pool(name="psB", bufs=4,
                                      space="PSUM") as psB:
                        for n in range(NQB):
                            nsl = slice(n * QP, (n + 1) * QP)
                            ps = psB.tile([128, QP], F32, tag="psB")
                            for k in range(KQ):
                                nc.tensor.matmul(
                                    ps, wqb[:, k, HPC * DN:HPC * DN + 128],
                                    qa_n[n][:, k, :],
                                    start=(k == 0), stop=(k == KQ - 1))
                            nc.any.tensor_copy(qrope[:, nsl], ps)

                        with tc.tile_pool(name="phc", bufs=1) as pc:
                            qxi = pc.tile([DR, SEQ], F16, tag="qxi")
                            nc.sync.dma_start(out=qxi, in_=qrope[64:128])
                            kpi = pc.tile([32, SEQ], F16, tag="kpi")
                            nc.sync.dma_start(out=kpi, in_=kpe[32:64])
                            yr64 = pc.tile([DR, SEQ], F16, tag="yr64")
                            yi64 = pc.tile([DR, SEQ], F16, tag="yi64")
                            kr32 = pc.tile([32, SEQ], F16, tag="kr32")
                            ki32 = pc.tile([32, SEQ], F16, tag="ki32")
                            for half in range(2):
                                hs = slice(half * SEQ // 2,
                                           (half + 1) * SEQ // 2)
                                qxr = qrope[0:64, hs]
                                qxih = qxi[:, hs]
                                c64, s64 = cost[:, hs], sint[:, hs]
                                ta = pc.tile([DR, SEQ // 2], F16,
                                             tag="ropetmp")
                                tb = pc.tile([DR, SEQ // 2], F16,
                                             tag="ropetmp2")
                                nc.vector.tensor_tensor(ta, qxr, c64, MM.mult)
                                nc.vector.tensor_tensor(tb, qxih, s64,
                                                        MM.mult)
                                nc.vector.tensor_tensor(yr64[:, hs], ta, tb,
                                                        MM.subtract)
                                ta = pc.tile([DR, SEQ // 2], F16,
                                             tag="ropetmp")
                                tb = pc.tile([DR, SEQ // 2], F16,
                                             tag="ropetmp2")
                                nc.vector.tensor_tensor(ta, qxr, s64, MM.mult)
                                nc.vector.tensor_tensor(tb, qxih, c64,
                                                        MM.mult)
                                nc.vector.tensor_tensor(yi64[:, hs], ta, tb,
                                                        MM.add)

                                kpr = kpe[0:32, hs]
                                kpih = kpi[:, hs]
                                c32, s32 = cost[0:32, hs], sint[0:32, hs]
                                ta = pc.tile([32, SEQ // 2], F16,
                                             tag="ropetmp")
                                tb = pc.tile([32, SEQ // 2], F16,
                                             tag="ropetmp2")
                                nc.vector.tensor_tensor(ta, kpr, c32, MM.mult)
                                nc.vector.tensor_tensor(tb, kpih, s32,
                                                        MM.mult)
                                nc.vector.tensor_tensor(kr32[:, hs], ta, tb,
                                                        MM.subtract)
                                ta = pc.tile([32, SEQ // 2], F16,
                                             tag="ropetmp")
                                tb = pc.tile([32, SEQ // 2], F16,
                                             tag="ropetmp2")
                                nc.vector.tensor_tensor(ta, kpr, s32, MM.mult)
                                nc.vector.tensor_tensor(tb, kpih, c32,
                                                        MM.mult)
                                nc.vector.tensor_tensor(ki32[:, hs], ta, tb,
                                                        MM.add)

                                for h in range(HPC):
                                    nc.sync.dma_start(
                                        out=q2[h][32:64, hs],
                                        in_=yr64[h * 32:(h + 1) * 32, hs])
                                    nc.sync.dma_start(
                                        out=q2[h][64:96, hs],
                                        in_=yi64[h * 32:(h + 1) * 32, hs])
                                    nc.sync.dma_start(out=k2[h][32:64, hs],
                                                      in_=kr32[:, hs])
                                    nc.sync.dma_start(out=k2[h][64:96, hs],
                                                      in_=ki32[:, hs])

                        # ==== phase B2 for q-block 0 (rest are fillers) ====
                        for h in range(HPC):
                            _emit_proj(nc, psB, wqb, wkvb, qa_n, kv_n,
                                       qnope, knope, vv, 0, h)

                    for h in range(HPC):
                        nc.sync.dma_start(out=q2[h][0:32, 0:QP],
                                          in_=qnope[h][96:128, 0:QP])
                        nc.sync.dma_start(out=k2[h][0:32, 0:QP],
                                          in_=knope[h][96:128, 0:QP])
                        nc.vector.tensor_scalar(vl[h][:, 0:4, :],
                                                vv[h][:, 0:4, :], float(lam),
                                                None, MM.mult)

                # ======== phases D + E + leftover B2, interleaved ========
                # Score tiles are (s1|s2) pairs in one 2-bank PSUM tile: one
                # exp, one mask multiply, one DVE z-accumulate per pair; the
                # denominators come from one final ones-matmul pair per
                # (head, q-block), column-tiled into partitions 0/32 of a
                # score-pool tile. Projections for q-blocks 1..3 and the
                # output projection are emitted as "fillers" inside the
                # m-loops so the PE stays dense while ACT runs the exps.
                with tc.tile_pool(name="pds", bufs=2, space="PSUM") as pds, \
                     tc.tile_pool(name="pdo", bufs=1, space="PSUM") as pdo, \
                     tc.tile_pool(name="mix", bufs=2, space="PSUM") as mix, \
                     tc.tile_pool(name="pde", bufs=3) as pde, \
                     tc.tile_pool(name="pdm", bufs=3) as pdm, \
                     tc.tile_pool(name="pda", bufs=2) as pda, \
                     tc.tile_pool(name="pdn", bufs=2) as pdn, \
                     tc.tile_pool(name="pey", bufs=2) as pey:
                    global _q2_of, _k2_of
                    _q2_of, _k2_of = q2, k2
                    fillers = []
                    for fn in range(1, NQB):
                        for fh in range(HPC):
                            fillers.append(("projq", fn, fh))
                            fillers.append(("projk", fn, fh))
                            fillers.append(("projv", fn, fh))

                    def emit_filler():
                        if not fillers:
                            return
                        kind, a1, a2 = fillers.pop(0)
                        if kind == "projq":
                            _emit_projq(nc, mix, wqb, qa_n, qnope, a1, a2)
                        elif kind == "projk":
                            _emit_projk(nc, mix, wkvb, kv_n, knope, a1, a2)
                        elif kind == "projv":
                            _emit_projv(nc, mix, wkvb, kv_n, vv, vl, lam,
                                        a1, a2)
                        else:
                            _emit_e(nc, mix, pey, wot, outT, yT, a1, a2)

                    for n in range(NQB):
                        m_list = [m for m in range(NKB)
                                  if block_kind[m][n] != 1]
                        for h in range(HPC):
                            op = pdo.tile([128, 2, QP], F32, tag="o")
                            acc = pda.tile([128, 2, QP], F16, tag="acc")
                            for i, m in enumerate(m_list):
                                first = (i == 0)
                                last = (i == len(m_list) - 1)
                                ksl = slice(m * KP, (m + 1) * KP)
                                d = (int(block_d[m][n])
                                     if block_kind[m][n] == 2 else 0)
                                act = QP - d
                                qsl = slice(n * QP + d, (n + 1) * QP)
                                sp = pds.tile([KP, 2, QP], F32, tag="s")
                                nc.tensor.matmul(sp[:, 0, d:],
                                                 knope[h][0:H, ksl],
                                                 qnope[h][0:H, qsl],
                                                 start=True, stop=True)
                                nc.tensor.matmul(sp[:, 1, d:], k2[h][:, ksl],
                                                 q2[h][:, qsl],
                                                 start=True, stop=True)
                                if first:
                                    emit_filler()
                                ep = pde.tile([KP, 2, QP], F16, tag="e")
                                nc.scalar.activation(ep[:, :, d:],
                                                     sp[:, :, d:], Exp)
                                if block_kind[m][n] == 2:
                                    mo = int(block_moff[m][n])
                                    xp = pdm.tile([KP, 2, QP], F16, tag="x")
                                    nc.vector.tensor_tensor(
                                        xp[:, :, d:], ep[:, :, d:],
                                        emt[:, mo:mo + 2 * act]
                                        .rearrange("p (t a) -> p t a", t=2),
                                        MM.mult)
                                    ep = xp
                                if first:
                                    nc.vector.tensor_copy(acc, ep)
                                else:
                                    nc.vector.tensor_tensor(
                                        acc[:, :, d:], acc[:, :, d:],
                                        ep[:, :, d:], MM.add)
                                nc.tensor.matmul(op[:, 0, d:], vv[h][:, m, :],
                                                 ep[:, 0, d:],
                                                 start=first, stop=last,
                                                 skip_group_check=True)
                                nc.tensor.matmul(op[:, 1, d:], vl[h][:, m, :],
                                                 ep[:, 1, d:],
                                                 start=first, stop=last,
                                                 skip_group_check=True)
                                if not first and i % 2 == 0:
                                    emit_filler()
                            # denominators + normalize:
                            #   outT = o1/z1 - lam*o2/z2 (lam folded into vl)
                            zt = pds.tile([KP, 2, QP], F32, tag="s")
                            nc.tensor.matmul(zt[0:1, 0, :], onest,
                                             acc[:, 0, :],
                                             start=True, stop=True,
                                             tile_position=(0, 0),
                                             skip_group_check=True)
                            nc.tensor.matmul(zt[32:33, 0, :], onest,
                                             acc[:, 1, :],
                                             start=True, stop=True,
                                             tile_position=(0, 32),
                                             skip_group_check=True)
                            r1 = pdn.tile([1, QP], F32, tag="r1")
                            nc.vector.reciprocal(r1, zt[0:1, 0, :])
                            r2 = pdn.tile([1, QP], F32, tag="r2")
                            nc.vector.reciprocal(r2, zt[32:33, 0, :])
                            rb = pdn.tile([128, 2 * QP], F32, tag="rb")
                            nc.gpsimd.partition_broadcast(rb[:, 0:QP], r1)
                            nc.gpsimd.partition_broadcast(rb[:, QP:], r2)
                            tp = pdn.tile([128, 2 * QP], F16, tag="tp")
                            nc.vector.tensor_tensor(
                                tp, op.rearrange("p t q -> p (t q)"), rb,
                                MM.mult)
                            nc.vector.tensor_tensor(
                                outT[h][:, n * QP:(n + 1) * QP],
                                tp[:, 0:QP], tp[:, QP:], MM.subtract)
                        for mog in range(DIM // 512):
                            fillers.append(("E", n, mog))
                    while fillers:
                        emit_filler()


def _emit_projq(nc, pool, wqb, qa_n, qnope, n, h):
    KQ = QLR // 128
    F32_ = mybir.dt.float32
    nsl = slice(n * QP, (n + 1) * QP)
    ps = pool.tile([128, QP], F32_, tag="mix", name="psq")
    for k in range(KQ):
        nc.tensor.matmul(ps, wqb[:, k, h * DN:(h + 1) * DN],
                         qa_n[n][:, k, :], start=(k == 0), stop=(k == KQ - 1))
    nc.any.tensor_copy(qnope[h][:, nsl], ps)
    nc.sync.dma_start(out=_q2_of[h][0:32, nsl], in_=qnope[h][96:128, nsl])


def _emit_projk(nc, pool, wkvb, kv_n, knope, n, h):
    KV = KVLR // 128
    F32_ = mybir.dt.float32
    nsl = slice(n * QP, (n + 1) * QP)
    co = h * (DN + DV)
    ps = pool.tile([128, QP], F32_, tag="mix", name="psk")
    for k in range(KV):
        nc.tensor.matmul(ps, wkvb[:, k, co:co + DN], kv_n[n][:, k, :],
                         start=(k == 0), stop=(k == KV - 1))
    nc.any.tensor_copy(knope[h][:, nsl], ps)
    nc.sync.dma_start(out=_k2_of[h][0:32, nsl], in_=knope[h][96:128, nsl])


def _emit_projv(nc, pool, wkvb, kv_n, vv, vl, lam, n, h):
    KV = KVLR // 128
    MM = mybir.AluOpType
    F32_ = mybir.dt.float32
    co = h * (DN + DV)
    psv = pool.tile([128, QP], F32_, tag="mix", name="psv") \
        .rearrange("p (c v) -> p c v", c=4)
    for smi in range(QP // 128):
        for k in range(KV):
            nc.tensor.matmul(psv[:, smi, :],
                             kv_n[n][:, k, smi * 128:(smi + 1) * 128],
                             wkvb[:, k, co + DN:co + DN + DV],
                             start=(k == 0), stop=(k == KV - 1),
                             skip_group_check=True)
    nc.any.tensor_copy(vv[h][:, 4 * n:4 * n + 4, :], psv)
    nc.vector.tensor_scalar(vl[h][:, 4 * n:4 * n + 4, :],
                            vv[h][:, 4 * n:4 * n + 4, :], float(lam),
                            None, MM.mult)


def _emit_e(nc, pool, pey, wot, outT, yT, n, mog):
    F32_ = mybir.dt.float32
    F16_ = mybir.dt.float16
    ty = pey.tile([128, 4, QP], F16_, tag="ty", name="ty")
    for j in range(4):
        mo = mog * 4 + j
        ps = pool.tile([128, QP], F32_, tag="mix", name="psE")
        for k in range(HPC):
            nc.tensor.matmul(ps, wot[:, k, mo * 128:(mo + 1) * 128],
                             outT[k][:, n * QP:(n + 1) * QP],
                             start=(k == 0), stop=(k == HPC - 1))
        nc.any.tensor_copy(ty[:, j, :], ps)
    nc.sync.dma_start(
        out=yT[mog * 512:(mog + 1) * 512, n * QP:(n + 1) * QP]
        .rearrange("(c p) s -> p c s", c=4),
        in_=ty)


def _emit_proj(nc, psB, wqb, wkvb, qa_n, kv_n, qnope, knope, vv, n, h):
    """Full projection set for (n, h) inside the phase-B pool."""
    KQ = QLR // 128
    KV = KVLR // 128
    F32_ = mybir.dt.float32
    nsl = slice(n * QP, (n + 1) * QP)
    ps = psB.tile([128, QP], F32_, tag="psB", name="psB")
    for k in range(KQ):
        nc.tensor.matmul(ps, wqb[:, k, h * DN:(h + 1) * DN],
                         qa_n[n][:, k, :], start=(k == 0), stop=(k == KQ - 1))
    nc.any.tensor_copy(qnope[h][:, nsl], ps)
    co = h * (DN + DV)
    ps = psB.tile([128, QP], F32_, tag="psB", name="psB2")
    for k in range(KV):
        nc.tensor.matmul(ps, wkvb[:, k, co:co + DN], kv_n[n][:, k, :],
                         start=(k == 0), stop=(k == KV - 1))
    nc.any.tensor_copy(knope[h][:, nsl], ps)
    psv = psB.tile([128, QP], F32_, tag="psB", name="psv") \
        .rearrange("p (c v) -> p c v", c=4)
    for smi in range(QP // 128):
        for k in range(KV):
            nc.tensor.matmul(psv[:, smi, :],
                             kv_n[n][:, k, smi * 128:(smi + 1) * 128],
                             wkvb[:, k, co + DN:co + DN + DV],
                             start=(k == 0), stop=(k == KV - 1),
                             skip_group_check=True)
    nc.any.tensor_copy(vv[h][:, 4 * n:4 * n + 4, :], psv)


def _build_nc(inputs, repeat=1):
    """Build both launch programs; returns (nc_a, nc_b, in_maps_a, in_maps_b)."""
    in_maps_a, in_maps_b, binfo = _host_prep(inputs)
    nc_a = bacc.Bacc("TRN2", target_bir_lowering=False, debug=False,
                     num_devices=N_CORES)
    _build_a(nc_a, repeat=repeat)
    nc_a.compile()
    nc_b = bacc.Bacc("TRN2", target_bir_lowering=False, debug=False,
                     num_devices=N_CORES)
    _build_be(nc_b, binfo, repeat=repeat)
    nc_b.compile()
    return nc_a, nc_b, in_maps_a, in_maps_b


def _gather_a(results_a):
    """Host gather of launch-1 outputs into full qa/kv/kpe arrays."""
    KQ = QLR // 128
    KV = KVLR // 128
    qa = np.empty((128, KQ, SEQ), np.float16)
    kv = np.empty((128, KV, SEQ), np.float16)
    kpe = np.empty((DR, SEQ), np.float16)
    for c, r in enumerate(results_a):
        sl = slice(c * SW, (c + 1) * SW)
        qa[:, :, sl] = r["qa_s"].reshape(128, KQ, SW)
        kv[:, :, sl] = r["kv_s"].reshape(128, KV, SW)
        kpe[:, sl] = r["kpe_s"]
    return (qa.reshape(128, -1), kv.reshape(128, -1),
            np.ascontiguousarray(kpe))


def kernel(**inputs):
    inputs = {k: np.asarray(v) for k, v in inputs.items()}
    nc_a, nc_b, in_maps_a, in_maps_b = _build_nc(inputs)
    res_a = run_bass_kernel_spmd(nc_a, in_maps_a,
                                 core_ids=list(range(N_CORES)))
    qa, kv, kpe = _gather_a(res_a.results)
    for m in in_maps_b:
        m["qa_d"] = qa
        m["kv_d"] = kv
        m["kpe_d"] = kpe
    res_b = run_bass_kernel_spmd(nc_b, in_maps_b,
                                 core_ids=list(range(N_CORES)))
    yT_sum = np.zeros((DIM, SEQ), np.float32)
    for r in res_b.results:
        yT_sum += r["yT"]
    return np.ascontiguousarray(yT_sum.T).reshape(1, SEQ, DIM).astype(np.float32)

